# revision 1
# baseline (speedup 1.0000x reference)
"""Multi-head attention kernel for Trainium2, data-parallel over 8 NeuronCores.

Problem: B=16, N=1024, D=768, H=12 heads (hd=64), fp32 I/O.
  qkv = x @ w_qkv + b_qkv ; attention ; out = attn_out @ w_proj + b_proj

Sharding: batch data-parallel — core c handles batches [2c, 2c+2); weights
replicated. Inside each core, the two batches are processed sequentially.

Layout strategy (all compute in f32r on TensorE — tf32-like, ~1.6e-4 rel):
  - host pre-transposes x to xT [768, T] so the in-feature contraction has
    features on partitions for both operands.
  - Q^T, K^T computed feature-major [768, N]: lhsT = w_qkv cols, rhs = xT.
    A 128-row feature tile holds a PAIR of heads (2x64) -> scores matmuls
    for the two heads run concurrently via tile_position row packing (K=64).
  - V computed token-major [N, 768]: lhsT = xT chunk, rhs = w_qkv v-cols,
    stored bf16 with a ones column appended per head (v_ext [128, 65]).
  - scores^T tiles [128 j, 512 q] per head -> one ACT exp op [128, 1024]
    covers both heads of a pair (softmax scale folded into exp's scale).
  - U^T = sum_j exp * v_ext accumulates in PSUM [65, 512]; row 64 is the
    softmax denominator (ones column) — no separate reduction needed.
  - normalize: reciprocal (DVE) -> partition_broadcast (GpSimd) -> multiply
    (DVE), + b_v per-partition. b_q/b_k added at Q^T/K^T evacuation
    (per-partition in feature-major layout).
  - proj: lhsT = attn^T tile, rhs = w_proj; + b_proj via broadcast add.
    Output lands token-major [T, 768] == final layout.
"""

import contextlib
import ctypes
import os
import sys
import types

import numpy as np

# ---------------------------------------------------------------------------
# NTFF profiling shim: bass_utils's trace path imports
# antenv.axon_hooks.get_axon_ntff_profile_hook, which this container's antenv
# lacks. Register a ctypes-based equivalent so BASS_TRACE=1 works. Harmless
# if tracing is never requested.
# ---------------------------------------------------------------------------


def _install_ntff_shim():
    if "antenv.axon_hooks" in sys.modules:
        return
    so_path = "/opt/axon/libaxon_pjrt.so"
    hook = None
    try:
        lib = ctypes.CDLL(so_path)
        if hasattr(lib, "axon_start_nrt_profile"):
            lib.axon_start_nrt_profile.argtypes = [
                ctypes.POINTER(ctypes.c_int64),
                ctypes.c_size_t,
            ]
            lib.axon_start_nrt_profile.restype = ctypes.c_int64
            lib.axon_stop_nrt_profile.argtypes = [ctypes.c_char_p]
            lib.axon_stop_nrt_profile.restype = ctypes.c_int64

            @contextlib.contextmanager
            def _hook(output_dir, device_ids):
                import jax

                jax.devices()
                if device_ids:
                    ids = (ctypes.c_int64 * len(device_ids))(*device_ids)
                    rc = lib.axon_start_nrt_profile(ids, len(device_ids))
                else:
                    rc = lib.axon_start_nrt_profile(None, 0)
                if rc != 0:
                    raise RuntimeError(f"axon_start_nrt_profile rc={rc}")
                try:
                    yield
                finally:
                    n = lib.axon_stop_nrt_profile(str(output_dir).encode())
                    print(f"ntff profile: {n} file(s) in {output_dir}",
                          file=sys.stderr)

            hook = _hook
    except OSError:
        pass
    mod = types.ModuleType("antenv.axon_hooks")
    mod.get_axon_ntff_profile_hook = lambda: hook
    mod.set_axon_ntff_profile_hook = lambda h: None
    sys.modules["antenv.axon_hooks"] = mod


_install_ntff_shim()

import concourse.bass_utils as _bu  # noqa: E402

if os.environ.get("LDW_OPT") == "1":
    _orig_run_command = _bu.run_command

    def _patched_run_command(argv, **kw):
        argv = [
            a.replace("--enable-ldw-opt=false", "--enable-ldw-opt=true")
            for a in argv
        ]
        return _orig_run_command(argv, **kw)

    _bu.run_command = _patched_run_command

import concourse.bacc as bacc  # noqa: E402
import concourse.mybir as mybir  # noqa: E402
import concourse.tile as tile  # noqa: E402
from concourse.bass_utils import run_bass_kernel_spmd  # noqa: E402

F32 = mybir.dt.float32
F32R = mybir.dt.float32r
BF16 = mybir.dt.bfloat16
AF = mybir.ActivationFunctionType

# Problem constants (per core)
NB = 2        # batches per core
TN = 1024     # tokens per batch
T = NB * TN   # tokens per core
D = 768
H = 12
HD = 64
D3 = 3 * D
KT = D // 128          # 6 contraction tiles
NPAIR = H // 2         # 6 head pairs
NJT = TN // 128        # 8 key tiles per batch
SCALE = HD ** -0.5


def build():
    nc = bacc.Bacc(None)
    xT_d = nc.declare_dram_parameter("xT", [D, T], BF16, isOutput=False)
    wqkv_d = nc.declare_dram_parameter("wqkv", [D, D3], BF16, isOutput=False)
    wproj_d = nc.declare_dram_parameter("wproj", [D, D], BF16, isOutput=False)
    bqk_d = nc.declare_dram_parameter("bqk", [128, 12], F32, isOutput=False)
    bv_d = nc.declare_dram_parameter("bv", [1, D], BF16, isOutput=False)
    bproj_d = nc.declare_dram_parameter("bproj", [1, D], BF16, isOutput=False)
    ones_d = nc.declare_dram_parameter("ones", [128, 96], BF16, isOutput=False)
    out_d = nc.declare_dram_parameter("out", [T, D], F32, isOutput=True)

    with tile.TileContext(nc) as tc:
        with (
            nc.allow_low_precision(reason="f32r/bf16 attention pipeline"),
            tc.tile_pool(name="const", bufs=1) as cpool,
            tc.tile_pool(name="xu", bufs=2) as xupool,
            tc.tile_pool(name="qk", bufs=2) as qkpool,
            tc.tile_pool(name="vsb", bufs=2) as vpool,
            tc.tile_pool(name="esb", bufs=5) as epool,
            tc.tile_pool(name="stg", bufs=8) as spool,
            tc.tile_pool(name="gat", bufs=2) as gpool,
            tc.tile_pool(name="gsh", bufs=3) as gspool,
            tc.tile_pool(name="bsb", bufs=2) as bpool,
            tc.tile_pool(name="osb", bufs=3) as opool,
            tc.tile_pool(name="psS", bufs=2, space="PSUM") as psS,
            tc.tile_pool(name="psU", bufs=2, space="PSUM") as psU,
            tc.tile_pool(name="psQ", bufs=2, space="PSUM") as psQ,
        ):
            # ---- constants / weights (resident) ----
            wqkv = cpool.tile([128, KT, D3], BF16, tag="wqkv")
            wqkv_src = wqkv_d.ap().rearrange("(ko p) n -> p ko n", p=128)
            # k-sliced so the first QK k-accumulation paces with the DMAs
            for k in range(KT):
                nc.sync.dma_start(
                    wqkv[:, k : k + 1, :], wqkv_src[:, k : k + 1, :]
                )
            wproj = cpool.tile([128, KT, D], BF16, tag="wproj")
            nc.sync.dma_start(
                wproj[:], wproj_d.ap().rearrange("(ko p) n -> p ko n", p=128)
            )
            bqk = cpool.tile([128, 12], F32, tag="bqk")
            nc.sync.dma_start(bqk[:], bqk_d.ap())
            bv1 = cpool.tile([1, D], BF16, tag="bv1")
            nc.sync.dma_start(bv1[:], bv_d.ap())
            bvb = cpool.tile([128, D], BF16, tag="bvb")
            nc.gpsimd.partition_broadcast(bvb[:], bv1[:])
            bproj1 = cpool.tile([1, D], BF16, tag="bproj1")
            nc.sync.dma_start(bproj1[:], bproj_d.ap())
            bprojb = cpool.tile([128, D], BF16, tag="bprojb")
            nc.gpsimd.partition_broadcast(bprojb[:], bproj1[:])

            prev_proj = None  # (uT, tok0) of the previous batch

            def emit_proj_tile(uT, tok0, t):
                for nh in range(2):
                    ps = psQ.tile([128, 384], F32, tag="ps")
                    for k in range(KT):
                        nc.tensor.matmul(
                            ps[:],
                            uT[:, k, t * 128 : (t + 1) * 128],
                            wproj[:, k, nh * 384 : (nh + 1) * 384],
                            start=(k == 0),
                            stop=(k == KT - 1),
                        )
                    ot = opool.tile([128, 384], F32, tag="o")
                    nc.vector.tensor_add(
                        ot[:], ps[:], bprojb[:, nh * 384 : (nh + 1) * 384]
                    )
                    nc.sync.dma_start(
                        out_d.ap()[
                            tok0 + t * 128 : tok0 + (t + 1) * 128,
                            nh * 384 : (nh + 1) * 384,
                        ],
                        ot[:],
                    )

            for b in range(NB):
                tok0 = b * TN
                qT = qkpool.tile([128, NPAIR, TN], BF16, tag="qT")
                kT = qkpool.tile([128, NPAIR, TN], BF16, tag="kT")
                vsb = vpool.tile([128, NJT, H, HD + 1], BF16, tag="v")
                nc.sync.dma_start(
                    vsb[:, :, :, HD : HD + 1],
                    ones_d.ap().rearrange("p (a b) -> p a b", a=NJT),
                )
                xTb = xupool.tile([128, KT, TN], BF16, tag="x", name=f"xT{b}")
                xT_src = xT_d.ap().rearrange("(ko p) n -> p ko n", p=128)[
                    :, :, tok0 : tok0 + TN
                ]
                for k in range(KT):
                    nc.sync.dma_start(
                        xTb[:, k : k + 1, :], xT_src[:, k : k + 1, :]
                    )
                uT = xupool.tile([128, KT, TN], BF16, tag="u", name=f"uT{b}")

                def emit_qk(hp):
                    # Q^T (m=hp) and K^T (m=hp+6) feature tiles for one pair
                    for m in (hp, hp + 6):
                        dst = qT if m < 6 else kT
                        for ih in range(2):
                            ps = psQ.tile([128, 512], F32, tag="ps")
                            for k in range(KT):
                                nc.tensor.matmul(
                                    ps[:],
                                    wqkv[:, k, m * 128 : (m + 1) * 128],
                                    xTb[:, k, ih * 512 : (ih + 1) * 512],
                                    start=(k == 0),
                                    stop=(k == KT - 1),
                                )
                            nc.vector.tensor_scalar_add(
                                dst[:, hp, ih * 512 : (ih + 1) * 512],
                                ps[:],
                                bqk[:, m : m + 1],
                            )

                def emit_v():
                    # V token-major into v_ext slots, + b_v (which then flows
                    # through exp@v_ext and the ones-column normalization)
                    for t in range(NJT):
                        for nh in range(2):
                            ps = psQ.tile([128, 384], F32, tag="ps")
                            for k in range(KT):
                                nc.tensor.matmul(
                                    ps[:],
                                    xTb[:, k, t * 128 : (t + 1) * 128],
                                    wqkv[
                                        :,
                                        k,
                                        2 * D + nh * 384 : 2 * D + (nh + 1) * 384,
                                    ],
                                    start=(k == 0),
                                    stop=(k == KT - 1),
                                )
                            nc.vector.tensor_add(
                                vsb[:, t, nh * 6 : (nh + 1) * 6, 0:HD],
                                ps[:],
                                bvb[:, nh * 384 : (nh + 1) * 384],
                            )

                def emit_norm(hp, stages):
                    """Normalize one head-pair: batched 4-row reciprocal at
                    partition bases 0/32/64/96 (one ~6cpe reciprocal covers
                    all four U tiles), then broadcast + multiply."""
                    g = gpool.tile([97, 512], F32, tag="g")
                    nc.vector.memset(g[:], 1.0)
                    for (ih, h, ust) in stages:
                        nc.vector.tensor_copy(
                            g[32 * (2 * ih + h) : 32 * (2 * ih + h) + 1, :],
                            ust[HD : HD + 1, :],
                        )
                    rc = gpool.tile([97, 512], F32, tag="g")
                    nc.vector.reciprocal(rc[:], g[:])
                    # partition_broadcast reads the tile's absolute partition
                    # 0 — shift rows 32/64/96 down first.
                    shifted = {0: rc}
                    for idx in (1, 2, 3):
                        tsh = gspool.tile([1, 512], F32, tag="gs", name=f"gs{idx}")
                        nc.vector.tensor_copy(
                            tsh[0:1, :], rc[32 * idx : 32 * idx + 1, :]
                        )
                        shifted[idx] = tsh
                    for (ih, h, ust) in stages:
                        idx = 2 * ih + h
                        rb = bpool.tile([128, 512], F32, tag="rb")
                        nc.gpsimd.partition_broadcast(rb[:], shifted[idx][0:1, :])
                        usl = uT[
                            h * 64 : (h + 1) * 64, hp, ih * 512 : ih * 512 + 512
                        ]
                        nc.vector.tensor_mul(usl, ust[0:HD, :], rb[0:HD, :])

                def emit_attn(hp, pending):
                    stages = []
                    for ih in range(2):
                        i0 = ih * 512
                        pu = [
                            psU.tile([HD + 1, 512], F32, tag="pu", name=f"pu{h}")
                            for h in range(2)
                        ]
                        for jt in range(NJT):
                            # scores^T for both heads, row-packed (K=64 each)
                            ps = psS.tile([128, 1024], F32, tag="s")
                            for h in range(2):
                                nc.tensor.matmul(
                                    ps[:, h * 512 : (h + 1) * 512],
                                    kT[
                                        h * 64 : (h + 1) * 64,
                                        hp,
                                        jt * 128 : (jt + 1) * 128,
                                    ],
                                    qT[h * 64 : (h + 1) * 64, hp, i0 : i0 + 512],
                                )
                            e = epool.tile([128, 1024], BF16, tag="e")
                            nc.scalar.activation(e[:], ps[:], AF.Exp, scale=SCALE)
                            for h in range(2):
                                nc.tensor.matmul(
                                    pu[h][:],
                                    vsb[:, jt, 2 * hp + h, :],
                                    e[:, h * 512 : (h + 1) * 512],
                                    start=(jt == 0),
                                    stop=(jt == NJT - 1),
                                )
                        # Only these two copies gate PSUM release; the slow
                        # normalization is deferred one pair so it never sits
                        # ahead of the next stage copies in DVE's queue.
                        for h in range(2):
                            ust = spool.tile([HD + 1, 512], F32, tag="ust")
                            if h == 0:
                                # split across ACT/DVE so both PSUM tiles
                                # release in parallel
                                nc.scalar.activation(
                                    ust[:], pu[h][:], AF.Copy
                                )
                            else:
                                nc.vector.tensor_copy(ust[:], pu[h][:])
                            stages.append((ih, h, ust))
                    if pending is not None:
                        emit_norm(*pending)
                    return (hp, stages)

                # Emission order drives per-engine execution order: keep ACT
                # fed by starting attention as soon as each pair's Q/K are
                # out, and slot the previous batch's projection tiles into
                # the gaps between attention pairs.
                emit_qk(0)
                emit_v()
                emit_qk(1)
                pending = None
                pt = 0  # previous-batch proj tiles emitted so far
                for hp in range(NPAIR):
                    pending = emit_attn(hp, pending)
                    if hp + 2 < NPAIR + 1:
                        emit_qk(hp + 2) if hp + 2 < NPAIR else None
                    if prev_proj is not None:
                        uTp, tok0p = prev_proj
                        for _ in range(2):
                            if pt < NJT:
                                emit_proj_tile(uTp, tok0p, pt)
                                pt += 1
                emit_norm(*pending)
                if prev_proj is not None:
                    uTp, tok0p = prev_proj
                    while pt < NJT:
                        emit_proj_tile(uTp, tok0p, pt)
                        pt += 1
                prev_proj = (uT, tok0)

            # final batch's projection
            uTp, tok0p = prev_proj
            for t in range(NJT):
                emit_proj_tile(uTp, tok0p, t)

    nc.compile()
    return nc


_NC_CACHE = None


def _get_nc():
    global _NC_CACHE
    if _NC_CACHE is None:
        _NC_CACHE = build()
    return _NC_CACHE


def _prep_core_inputs(x_c, w_qkv, b_qkv, w_proj, b_proj):
    """Host-side layout prep for one core. x_c: [2, 1024, 768]."""
    xT = np.ascontiguousarray(x_c.reshape(T, D).T).astype(np.float32)
    bqk = np.ascontiguousarray(b_qkv[: 12 * 128].reshape(12, 128).T)
    import ml_dtypes

    bf = ml_dtypes.bfloat16
    return {
        "xT": np.ascontiguousarray(xT.astype(bf)),
        "wqkv": np.ascontiguousarray(w_qkv.astype(bf)),
        "wproj": np.ascontiguousarray(w_proj.astype(bf)),
        "bqk": bqk.astype(np.float32),
        "bv": np.ascontiguousarray(b_qkv[2 * D :].reshape(1, D).astype(bf)),
        "bproj": np.ascontiguousarray(b_proj.reshape(1, D).astype(bf)),
        "ones": np.ones((128, 96), dtype=bf),
    }


def kernel(x, w_qkv, b_qkv, w_proj, b_proj):
    x = np.asarray(x, dtype=np.float32)
    w_qkv = np.asarray(w_qkv, dtype=np.float32)
    b_qkv = np.asarray(b_qkv, dtype=np.float32)
    w_proj = np.asarray(w_proj, dtype=np.float32)
    b_proj = np.asarray(b_proj, dtype=np.float32)
    B, N, Dd = x.shape
    assert (B, N, Dd) == (16, 1024, 768)

    nc = _get_nc()
    in_maps = [
        _prep_core_inputs(x[2 * c : 2 * c + 2], w_qkv, b_qkv, w_proj, b_proj)
        for c in range(8)
    ]
    res = run_bass_kernel_spmd(nc, in_maps, core_ids=list(range(8)))
    out = np.empty((B, N, Dd), dtype=np.float32)
    for c in range(8):
        out[2 * c : 2 * c + 2] = res.results[c]["out"].reshape(2, N, Dd)
    kernel.last_results = res
    return out



# revision 14
# speedup vs baseline: 1.1508x; 1.1508x over previous
"""Multi-head attention kernel for Trainium2, data-parallel over 8 NeuronCores.

Problem: B=16, N=1024, D=768, H=12 heads (hd=64), fp32 I/O.
  qkv = x @ w_qkv + b_qkv ; attention ; out = attn_out @ w_proj + b_proj

Sharding: batch data-parallel — core c handles batches [2c, 2c+2); weights
replicated. Inside each core, the two batches are processed sequentially.

Layout strategy (matmuls in bf16, ~7e-3 rel overall):
  - host pre-transposes x to xT [768, T] so the in-feature contraction has
    features on partitions for both operands.
  - Q^T, K^T computed feature-major [768, N]: lhsT = w_qkv cols, rhs = xT.
    A 128-row feature tile holds a PAIR of heads (2x64) -> scores matmuls
    for the two heads share one PSUM tile via row packing (K=64).
  - V computed token-major [N, 768], stored bf16 with a ones column
    appended per head (v_ext [128, 65]).
  - scores^T tiles [128 j, 512 q] per head -> one ACT exp op [128, 1024]
    covers both heads of a pair (softmax scale folded into exp's scale).
  - U^T = sum_j exp * v_ext accumulates in PSUM [65, 512]; row 64 is the
    softmax denominator (ones column) — no separate reduction needed.
  - normalize: reciprocal (DVE) -> partition_broadcast (GpSimd) -> multiply
    (DVE), + b_v per-partition. b_q/b_k added at Q^T/K^T evacuation.
  - proj: lhsT = attn^T tile, rhs = w_proj; + b_proj via broadcast add.
    Output lands token-major [T, 768] in bf16 (upcast on host).

Schedule (the point of this revision):
  - ACT exp (1063ns per [128,1024]) paces each attention step; the two
    scores + two attnV matmuls only cover 852ns. A queue of single-matmul
    "filler" closures (QK / V / proj tiles) is popped between attention
    steps so TensorE never idles waiting for exp.
  - DMA priority order: bqk, xT(b0), wqkv k-slices, bv, bproj, wproj,
    xT(b1) — the first QK matmul can start as soon as wqkv slice k=0
    lands (~7us) instead of after all inputs (~32us).
  - Dummy matmuls on a zeroed tile bridge the initial DMA wait so the PE
    HAM clock-gate warms up (and stays warm — no 1.2GHz cold periods).
  - ones column of v_ext via on-chip memset (the DMA version generated
    24576 2-byte packets that polluted every DMA queue).
  - Endgame: the last pair's normalization runs via ACT reciprocal +
    TensorE ones-broadcast (short latency, engines that are idle then),
    and the final batch's projection is split into two token halves so
    the first half overlaps the last pair's ih=1 attention.
"""

import contextlib
import ctypes
import os
import sys
import types

import numpy as np

# ---------------------------------------------------------------------------
# NTFF profiling shim: bass_utils's trace path imports
# antenv.axon_hooks.get_axon_ntff_profile_hook, which this container's antenv
# lacks. Register a ctypes-based equivalent so BASS_TRACE=1 works. Harmless
# if tracing is never requested.
# ---------------------------------------------------------------------------


def _install_ntff_shim():
    if "antenv.axon_hooks" in sys.modules:
        return
    so_path = "/opt/axon/libaxon_pjrt.so"
    hook = None
    try:
        lib = ctypes.CDLL(so_path)
        if hasattr(lib, "axon_start_nrt_profile"):
            lib.axon_start_nrt_profile.argtypes = [
                ctypes.POINTER(ctypes.c_int64),
                ctypes.c_size_t,
            ]
            lib.axon_start_nrt_profile.restype = ctypes.c_int64
            lib.axon_stop_nrt_profile.argtypes = [ctypes.c_char_p]
            lib.axon_stop_nrt_profile.restype = ctypes.c_int64

            @contextlib.contextmanager
            def _hook(output_dir, device_ids):
                import jax

                jax.devices()
                if device_ids:
                    ids = (ctypes.c_int64 * len(device_ids))(*device_ids)
                    rc = lib.axon_start_nrt_profile(ids, len(device_ids))
                else:
                    rc = lib.axon_start_nrt_profile(None, 0)
                if rc != 0:
                    raise RuntimeError(f"axon_start_nrt_profile rc={rc}")
                try:
                    yield
                finally:
                    n = lib.axon_stop_nrt_profile(str(output_dir).encode())
                    print(f"ntff profile: {n} file(s) in {output_dir}",
                          file=sys.stderr)

            hook = _hook
    except OSError:
        pass
    mod = types.ModuleType("antenv.axon_hooks")
    mod.get_axon_ntff_profile_hook = lambda: hook
    mod.set_axon_ntff_profile_hook = lambda h: None
    sys.modules["antenv.axon_hooks"] = mod


_install_ntff_shim()

import concourse.bacc as bacc  # noqa: E402
import concourse.mybir as mybir  # noqa: E402
import concourse.tile as tile  # noqa: E402
from concourse.bass_utils import run_bass_kernel_spmd  # noqa: E402

F32 = mybir.dt.float32
F32R = mybir.dt.float32r
BF16 = mybir.dt.bfloat16
AF = mybir.ActivationFunctionType

# Problem constants (per core)
NB = 2        # batches per core
TN = 1024     # tokens per batch
T = NB * TN   # tokens per core
D = 768
H = 12
HD = 64
D3 = 3 * D
KT = D // 128          # 6 contraction tiles
NPAIR = H // 2         # 6 head pairs
NJT = TN // 128        # 8 key tiles per batch
SCALE = HD ** -0.5


def build():
    nc = bacc.Bacc(None)
    xT_d = nc.declare_dram_parameter("xT", [D, T], BF16, isOutput=False)
    wqkv_d = nc.declare_dram_parameter("wqkv", [D, D3], BF16, isOutput=False)
    wproj_d = nc.declare_dram_parameter("wproj", [D, D], BF16, isOutput=False)
    bqk_d = nc.declare_dram_parameter("bqk", [128, 12], F32, isOutput=False)
    bv_d = nc.declare_dram_parameter("bv", [1, D], BF16, isOutput=False)
    bproj_d = nc.declare_dram_parameter("bproj", [1, D], BF16, isOutput=False)
    out_d = nc.declare_dram_parameter("out", [T, D], BF16, isOutput=True)

    with tile.TileContext(nc) as tc:
        with (
            nc.allow_low_precision(reason="bf16 attention pipeline"),
            tc.tile_pool(name="const", bufs=1) as cpool,
            tc.tile_pool(name="xu", bufs=2) as xupool,
            tc.tile_pool(name="qk", bufs=2) as qkpool,
            tc.tile_pool(name="vsb", bufs=2) as vpool,
            tc.tile_pool(name="esb", bufs=4) as epool,
            tc.tile_pool(name="stg", bufs=6) as spool,
            tc.tile_pool(name="gat", bufs=2) as gpool,
            tc.tile_pool(name="gsh", bufs=3) as gspool,
            tc.tile_pool(name="bsb", bufs=2) as bpool,
            tc.tile_pool(name="osb", bufs=3) as opool,
            tc.tile_pool(name="psS", bufs=2, space="PSUM") as psS,
            tc.tile_pool(name="psU", bufs=2, space="PSUM") as psU,
            tc.tile_pool(name="psQ", bufs=2, space="PSUM") as psQ,
        ):
            # ---- DMAs in priority order ----
            bqk = cpool.tile([128, 12], F32, tag="bqk")
            nc.sync.dma_start(bqk[:], bqk_d.ap())

            xTb = [None, None]
            xT_src = xT_d.ap().rearrange("(ko p) n -> p ko n", p=128)
            xTb[0] = xupool.tile([128, KT, TN], BF16, tag="x", name="xT0")
            for k in range(KT):
                nc.sync.dma_start(
                    xTb[0][:, k : k + 1, :], xT_src[:, k : k + 1, 0:TN]
                )

            wqkv = cpool.tile([128, KT, D3], BF16, tag="wqkv")
            wqkv_src = wqkv_d.ap().rearrange("(ko p) n -> p ko n", p=128)
            for k in range(KT):
                nc.sync.dma_start(
                    wqkv[:, k : k + 1, :], wqkv_src[:, k : k + 1, :]
                )

            bv1 = cpool.tile([1, D], BF16, tag="bv1")
            nc.sync.dma_start(bv1[:], bv_d.ap())
            bproj1 = cpool.tile([1, D], BF16, tag="bproj1")
            nc.sync.dma_start(bproj1[:], bproj_d.ap())
            wproj = cpool.tile([128, KT, D], BF16, tag="wproj")
            nc.sync.dma_start(
                wproj[:], wproj_d.ap().rearrange("(ko p) n -> p ko n", p=128)
            )
            xTb[1] = xupool.tile([128, KT, TN], BF16, tag="x", name="xT1")
            for k in range(KT):
                nc.sync.dma_start(
                    xTb[1][:, k : k + 1, :], xT_src[:, k : k + 1, TN : 2 * TN]
                )

            # ---- warmup: zeroed matmuls bridge the DMA wait so the PE
            # HAM clock-gate is warm when real work starts ----
            zt = cpool.tile([128, 640], BF16, tag="zt")
            nc.vector.memset(zt[:], 0.0)
            for _ in range(12):
                ps = psQ.tile([128, 512], F32, tag="ps")
                nc.tensor.matmul(
                    ps[:], zt[:, 0:128], zt[:, 128:640], start=True, stop=True
                )

            bvb = cpool.tile([128, D], BF16, tag="bvb")
            nc.gpsimd.partition_broadcast(bvb[:], bv1[:])
            bprojb = cpool.tile([128, D], BF16, tag="bprojb")
            nc.gpsimd.partition_broadcast(bprojb[:], bproj1[:])

            # ---- per-batch tiles (created lazily) ----
            qT = [None, None]
            kT = [None, None]
            vsb = [None, None]
            uT = [None, None]

            def make_batch_tiles(b):
                qT[b] = qkpool.tile([128, NPAIR, TN], BF16, tag="qT",
                                    name=f"qT{b}")
                kT[b] = qkpool.tile([128, NPAIR, TN], BF16, tag="kT",
                                    name=f"kT{b}")
                uT[b] = xupool.tile([128, KT, TN], BF16, tag="u",
                                    name=f"uT{b}")

            # ---- filler queue machinery ----
            # each item: (closure, tag_or_None); tag marks completion of a
            # unit of work other emission must wait for (emission order is
            # the only dependency mechanism — a read emitted before its
            # producer would silently read stale data).
            from collections import deque

            queue = deque()
            emitted = set()

            def pop(n=1):
                for _ in range(n):
                    if not queue:
                        return
                    fn, tag = queue.popleft()
                    fn()
                    if tag is not None:
                        emitted.add(tag)

            def require(tag):
                while tag not in emitted and queue:
                    pop(1)
                assert tag in emitted, f"filler ordering bug: {tag}"

            def qk_items(b, hp):
                """24 matmul closures computing Q^T,K^T feature tiles of
                head-pair hp; DVE bias-add evac attached to each 6th."""
                items = []
                for m in (hp, hp + 6):
                    for ih in range(2):
                        st = {}

                        def mk(k, m=m, ih=ih, st=st):
                            def f():
                                if k == 0:
                                    st["ps"] = psQ.tile([128, 512], F32,
                                                        tag="ps", name="ps")
                                dst = qT[b] if m < 6 else kT[b]
                                nc.tensor.matmul(
                                    st["ps"][:],
                                    wqkv[:, k, m * 128 : (m + 1) * 128],
                                    xTb[b][:, k, ih * 512 : (ih + 1) * 512],
                                    start=(k == 0),
                                    stop=(k == KT - 1),
                                )
                                if k == KT - 1:
                                    nc.vector.tensor_scalar_add(
                                        dst[:, hp, ih * 512 : (ih + 1) * 512],
                                        st["ps"][:],
                                        bqk[:, m : m + 1],
                                    )
                            return f

                        for k in range(KT):
                            items.append((mk(k), None))
                items[-1] = (items[-1][0], ("qk", b, hp))
                return items

            def v_items(b, nh, ts):
                """V tiles (token-major, + b_v) for key tiles ts; tag per
                completed (b, nh, t) tile."""
                items = []
                for t in ts:
                    st = {}

                    def mk(k, t=t, st=st):
                        def f():
                            if k == 0:
                                st["ps"] = psQ.tile([128, 384], F32,
                                                    tag="ps", name="ps")
                            nc.tensor.matmul(
                                st["ps"][:],
                                xTb[b][:, k, t * 128 : (t + 1) * 128],
                                wqkv[
                                    :, k,
                                    2 * D + nh * 384 : 2 * D + (nh + 1) * 384,
                                ],
                                start=(k == 0),
                                stop=(k == KT - 1),
                            )
                            if k == KT - 1:
                                nc.vector.tensor_add(
                                    vsb[b][:, t, nh * 6 : (nh + 1) * 6, 0:HD],
                                    st["ps"][:],
                                    bvb[:, nh * 384 : (nh + 1) * 384],
                                )
                        return f

                    for k in range(KT):
                        items.append(
                            (mk(k), ("v", b, nh, t) if k == KT - 1 else None)
                        )
                return items

            def vsb_init_item(b):
                def f():
                    vsb[b] = vpool.tile([128, NJT, H, HD + 1], BF16,
                                        tag="v", name=f"v{b}")
                    nc.vector.memset(vsb[b][:, :, :, HD : HD + 1], 1.0)
                return (f, ("vsb", b))

            def proj_items(b, ts):
                """Projection tiles for batch b tokens ts (needs uT[b]
                fully normalized); bf16 out + DMA."""
                tok0 = b * TN
                items = []
                for t in ts:
                    for nh in range(2):
                        st = {}

                        def mk(k, t=t, nh=nh, st=st):
                            def f():
                                if k == 0:
                                    st["ps"] = psQ.tile([128, 384], F32,
                                                        tag="ps", name="ps")
                                nc.tensor.matmul(
                                    st["ps"][:],
                                    uT[b][:, k, t * 128 : (t + 1) * 128],
                                    wproj[:, k, nh * 384 : (nh + 1) * 384],
                                    start=(k == 0),
                                    stop=(k == KT - 1),
                                )
                                if k == KT - 1:
                                    ot = opool.tile([128, 384], BF16,
                                                    tag="o", name="ot")
                                    nc.vector.tensor_add(
                                        ot[:], st["ps"][:],
                                        bprojb[:, nh * 384 : (nh + 1) * 384],
                                    )
                                    nc.sync.dma_start(
                                        out_d.ap()[
                                            tok0 + t * 128 : tok0 + (t + 1) * 128,
                                            nh * 384 : (nh + 1) * 384,
                                        ],
                                        ot[:],
                                    )
                            return f

                        for k in range(KT):
                            items.append((mk(k), None))
                items[-1] = (items[-1][0], ("proj", b, ts[-1]))
                return items

            # ---- normalization ----
            def emit_norm(b, hp, stages):
                """Batched normalization of one head-pair (4 (ih,h) stages):
                one reciprocal covers all four denominator rows (at
                partition bases 0/32/64/96), then GpSimd broadcast + DVE
                multiply. Runs off the critical path."""
                g = gpool.tile([97, 512], F32, tag="g")
                nc.vector.memset(g[:], 1.0)
                for (ih, h, ust) in stages:
                    nc.vector.tensor_copy(
                        g[32 * (2 * ih + h) : 32 * (2 * ih + h) + 1, :],
                        ust[HD : HD + 1, :],
                    )
                rc = gpool.tile([97, 512], F32, tag="g")
                # denominators are sums of exps (positive, ~e^±8): safe for
                # the fast approx (~18 correct bits, 5x cheaper), and the
                # result feeds bf16 anyway.
                nc.vector.reciprocal_approx_fast(rc[:], g[:])
                shifted = {0: rc}
                for idx in (1, 2, 3):
                    tsh = gspool.tile([1, 512], F32, tag="gs",
                                      name=f"gs{idx}")
                    nc.vector.tensor_copy(
                        tsh[0:1, :], rc[32 * idx : 32 * idx + 1, :]
                    )
                    shifted[idx] = tsh
                for (ih, h, ust) in stages:
                    idx = 2 * ih + h
                    rb = bpool.tile([128, 512], F32, tag="rb")
                    nc.gpsimd.partition_broadcast(rb[:], shifted[idx][0:1, :])
                    usl = uT[b][
                        h * 64 : (h + 1) * 64, hp, ih * 512 : ih * 512 + 512
                    ]
                    nc.vector.tensor_mul(usl, ust[0:HD, :], rb[0:HD, :])

            def emit_norm_fast(b, hp, stages):
                """Endgame normalization of a 2-stage (single ih) subset —
                same structure as emit_norm; latency is short because only
                two denominator rows ride the reciprocal."""
                emit_norm(b, hp, stages)

            # ---- paced attention pair ----
            pending = [None]  # (b, hp, stages) awaiting normalization

            def emit_attn(b, hp, endgame=False):
                if pending[0] is not None:
                    emit_norm(*pending[0])
                    pending[0] = None
                nh = 0 if hp < 3 else 1
                stages = []
                for ih in range(2):
                    i0 = ih * 512
                    pu = [
                        psU.tile([HD + 1, 512], F32, tag="pu", name=f"pu{h}")
                        for h in range(2)
                    ]
                    e_tiles = []
                    for jt in range(NJT):
                        ps = psS.tile([128, 1024], F32, tag="s")
                        for h in range(2):
                            nc.tensor.matmul(
                                ps[:, h * 512 : (h + 1) * 512],
                                kT[b][h * 64 : (h + 1) * 64, hp,
                                      jt * 128 : (jt + 1) * 128],
                                qT[b][h * 64 : (h + 1) * 64, hp, i0 : i0 + 512],
                            )
                        e = epool.tile([128, 1024], BF16, tag="e")
                        nc.scalar.activation(e[:], ps[:], AF.Exp, scale=SCALE)
                        e_tiles.append(e)
                        if jt == 0:
                            pop(2)
                        else:
                            require(("v", b, nh, jt - 1))
                            for h in range(2):
                                nc.tensor.matmul(
                                    pu[h][:],
                                    vsb[b][:, jt - 1, 2 * hp + h, :],
                                    e_tiles[jt - 1][:, h * 512 : (h + 1) * 512],
                                    start=(jt == 1),
                                    stop=False,
                                )
                            pop(2)
                    require(("v", b, nh, NJT - 1))
                    for h in range(2):
                        nc.tensor.matmul(
                            pu[h][:],
                            vsb[b][:, NJT - 1, 2 * hp + h, :],
                            e_tiles[NJT - 1][:, h * 512 : (h + 1) * 512],
                            start=False,
                            stop=True,
                        )
                    ih_stages = []
                    for h in range(2):
                        ust = spool.tile([HD + 1, 512], F32, tag="ust")
                        nc.vector.tensor_copy(ust[:], pu[h][:])
                        ih_stages.append((ih, h, ust))
                    stages += ih_stages
                    if endgame and ih == 0:
                        # normalize ih0 of the final pair right away (fast
                        # path) and queue the first half of the final
                        # projection as fillers for ih1.
                        emit_norm_fast(b, hp, ih_stages)
                        queue.extend(proj_items(b, [0, 1, 2, 3]))
                if endgame:
                    emit_norm_fast(b, hp, stages[2:])
                else:
                    pending[0] = (b, hp, stages)

            # =================================================================
            # batch 0
            # =================================================================
            make_batch_tiles(0)
            f, tag = vsb_init_item(0)
            f()
            emitted.add(tag)
            for fn, tag in qk_items(0, 0):
                fn()
                if tag:
                    emitted.add(tag)
            for fn, tag in v_items(0, 0, [0, 1, 2, 3]):
                fn()
                if tag:
                    emitted.add(tag)

            queue.extend(v_items(0, 0, [4, 5, 6, 7]))
            queue.extend(qk_items(0, 1))
            queue.extend(qk_items(0, 2))
            queue.extend(v_items(0, 1, list(range(NJT))))
            queue.extend(qk_items(0, 3))
            queue.extend(qk_items(0, 4))
            queue.extend(qk_items(0, 5))
            # cross-batch lead-in: batch 1's first QK pair + early V tiles
            # fill batch 0's last attention pairs.
            def b1_tiles_item():
                def f():
                    make_batch_tiles(1)
                return (f, ("tiles", 1))

            queue.append(b1_tiles_item())
            queue.append(vsb_init_item(1))
            queue.extend(qk_items(1, 0))
            queue.extend(v_items(1, 0, [0, 1, 2, 3]))
            queue.extend(qk_items(1, 1))

            for hp in range(NPAIR):
                require(("qk", 0, hp))
                emit_attn(0, hp)

            # =================================================================
            # batch 1
            # =================================================================
            require(("tiles", 1))
            require(("vsb", 1))
            if pending[0] is not None:
                emit_norm(*pending[0])
                pending[0] = None

            queue.extend(v_items(1, 0, [4, 5, 6, 7]))
            queue.extend(qk_items(1, 2))
            queue.extend(v_items(1, 1, list(range(NJT))))
            queue.extend(qk_items(1, 3))
            queue.extend(proj_items(0, [0, 1, 2, 3]))
            queue.extend(qk_items(1, 4))
            queue.extend(proj_items(0, [4, 5, 6, 7]))
            queue.extend(qk_items(1, 5))

            for hp in range(NPAIR):
                require(("qk", 1, hp))
                if hp == NPAIR - 1:
                    # the tail must be only the final projection: drain
                    # everything else (incl. batch 0's proj) first.
                    require(("proj", 0, 7))
                emit_attn(1, hp, endgame=(hp == NPAIR - 1))

            # final projection, second token half (first half was queued
            # during the endgame pair's ih=1)
            queue.extend(proj_items(1, [4, 5, 6, 7]))
            while queue:
                pop(1)

    nc.compile()
    return nc


_NC_CACHE = None


def _get_nc():
    global _NC_CACHE
    if _NC_CACHE is None:
        _NC_CACHE = build()
    return _NC_CACHE


def _prep_core_inputs(x_c, w_qkv, b_qkv, w_proj, b_proj):
    """Host-side layout prep for one core. x_c: [2, 1024, 768]."""
    xT = np.ascontiguousarray(x_c.reshape(T, D).T).astype(np.float32)
    bqk = np.ascontiguousarray(b_qkv[: 12 * 128].reshape(12, 128).T)
    import ml_dtypes

    bf = ml_dtypes.bfloat16
    return {
        "xT": np.ascontiguousarray(xT.astype(bf)),
        "wqkv": np.ascontiguousarray(w_qkv.astype(bf)),
        "wproj": np.ascontiguousarray(w_proj.astype(bf)),
        "bqk": bqk.astype(np.float32),
        "bv": np.ascontiguousarray(b_qkv[2 * D :].reshape(1, D).astype(bf)),
        "bproj": np.ascontiguousarray(b_proj.reshape(1, D).astype(bf)),
    }


def kernel(x, w_qkv, b_qkv, w_proj, b_proj):
    x = np.asarray(x, dtype=np.float32)
    w_qkv = np.asarray(w_qkv, dtype=np.float32)
    b_qkv = np.asarray(b_qkv, dtype=np.float32)
    w_proj = np.asarray(w_proj, dtype=np.float32)
    b_proj = np.asarray(b_proj, dtype=np.float32)
    B, N, Dd = x.shape
    assert (B, N, Dd) == (16, 1024, 768)

    nc = _get_nc()
    in_maps = [
        _prep_core_inputs(x[2 * c : 2 * c + 2], w_qkv, b_qkv, w_proj, b_proj)
        for c in range(8)
    ]
    res = run_bass_kernel_spmd(nc, in_maps, core_ids=list(range(8)))
    out = np.empty((B, N, Dd), dtype=np.float32)
    for c in range(8):
        out[2 * c : 2 * c + 2] = (
            res.results[c]["out"].astype(np.float32).reshape(2, N, Dd)
        )
    kernel.last_results = res
    return out


# revision 18
# speedup vs baseline: 1.1773x; 1.0230x over previous
"""Multi-head attention kernel for Trainium2, data-parallel over 8 NeuronCores.

Problem: B=16, N=1024, D=768, H=12 heads (hd=64), fp32 I/O.
  qkv = x @ w_qkv + b_qkv ; attention ; out = attn_out @ w_proj + b_proj

Sharding: batch data-parallel — core c handles batches [2c, 2c+2); weights
replicated. Inside each core, the two batches are processed sequentially.

Layout strategy (matmuls in bf16, ~7e-3 rel overall):
  - host pre-transposes x to xT [768, T] so the in-feature contraction has
    features on partitions for both operands.
  - Q^T, K^T computed feature-major [768, N]: lhsT = w_qkv cols, rhs = xT.
    A 128-row feature tile holds a PAIR of heads (2x64) -> scores matmuls
    for the two heads share one PSUM tile via row packing (K=64).
  - V computed token-major [N, 768], stored bf16 with a ones column
    appended per head (v_ext [128, 65]).
  - scores^T tiles [128 j, 512 q] per head -> one ACT exp op [128, 1024]
    covers both heads of a pair (softmax scale folded into exp's scale).
  - U^T = sum_j exp * v_ext accumulates in PSUM [65, 512]; row 64 is the
    softmax denominator (ones column) — no separate reduction needed.
  - normalize: reciprocal (DVE) -> partition_broadcast (GpSimd) -> multiply
    (DVE), + b_v per-partition. b_q/b_k added at Q^T/K^T evacuation.
  - proj: lhsT = attn^T tile, rhs = w_proj; + b_proj via broadcast add.
    Output lands token-major [T, 768] in bf16 (upcast on host).

Schedule (the point of this revision):
  - ACT exp (1063ns per [128,1024]) paces each attention step; the two
    scores + two attnV matmuls only cover 852ns. A queue of single-matmul
    "filler" closures (QK / V / proj tiles) is popped between attention
    steps so TensorE never idles waiting for exp.
  - DMA priority order: bqk, xT(b0), wqkv k-slices, bv, bproj, wproj,
    xT(b1) — the first QK matmul can start as soon as wqkv slice k=0
    lands (~7us) instead of after all inputs (~32us).
  - Dummy matmuls on a zeroed tile bridge the initial DMA wait so the PE
    HAM clock-gate warms up (and stays warm — no 1.2GHz cold periods).
  - ones column of v_ext via on-chip memset (the DMA version generated
    24576 2-byte packets that polluted every DMA queue).
  - Endgame: the last pair's normalization runs via ACT reciprocal +
    TensorE ones-broadcast (short latency, engines that are idle then),
    and the final batch's projection is split into two token halves so
    the first half overlaps the last pair's ih=1 attention.
"""

import contextlib
import ctypes
import os
import sys
import types

import numpy as np

# ---------------------------------------------------------------------------
# NTFF profiling shim: bass_utils's trace path imports
# antenv.axon_hooks.get_axon_ntff_profile_hook, which this container's antenv
# lacks. Register a ctypes-based equivalent so BASS_TRACE=1 works. Harmless
# if tracing is never requested.
# ---------------------------------------------------------------------------


def _install_ntff_shim():
    if "antenv.axon_hooks" in sys.modules:
        return
    so_path = "/opt/axon/libaxon_pjrt.so"
    hook = None
    try:
        lib = ctypes.CDLL(so_path)
        if hasattr(lib, "axon_start_nrt_profile"):
            lib.axon_start_nrt_profile.argtypes = [
                ctypes.POINTER(ctypes.c_int64),
                ctypes.c_size_t,
            ]
            lib.axon_start_nrt_profile.restype = ctypes.c_int64
            lib.axon_stop_nrt_profile.argtypes = [ctypes.c_char_p]
            lib.axon_stop_nrt_profile.restype = ctypes.c_int64

            @contextlib.contextmanager
            def _hook(output_dir, device_ids):
                import jax

                jax.devices()
                if device_ids:
                    ids = (ctypes.c_int64 * len(device_ids))(*device_ids)
                    rc = lib.axon_start_nrt_profile(ids, len(device_ids))
                else:
                    rc = lib.axon_start_nrt_profile(None, 0)
                if rc != 0:
                    raise RuntimeError(f"axon_start_nrt_profile rc={rc}")
                try:
                    yield
                finally:
                    n = lib.axon_stop_nrt_profile(str(output_dir).encode())
                    print(f"ntff profile: {n} file(s) in {output_dir}",
                          file=sys.stderr)

            hook = _hook
    except OSError:
        pass
    mod = types.ModuleType("antenv.axon_hooks")
    mod.get_axon_ntff_profile_hook = lambda: hook
    mod.set_axon_ntff_profile_hook = lambda h: None
    sys.modules["antenv.axon_hooks"] = mod


_install_ntff_shim()

import concourse.bacc as bacc  # noqa: E402
import concourse.mybir as mybir  # noqa: E402
import concourse.tile as tile  # noqa: E402
from concourse.bass_utils import run_bass_kernel_spmd  # noqa: E402

F32 = mybir.dt.float32
F32R = mybir.dt.float32r
BF16 = mybir.dt.bfloat16
AF = mybir.ActivationFunctionType

# Problem constants (per core)
NB = 2        # batches per core
TN = 1024     # tokens per batch
T = NB * TN   # tokens per core
D = 768
H = 12
HD = 64
D3 = 3 * D
KT = D // 128          # 6 contraction tiles
NPAIR = H // 2         # 6 head pairs
NJT = TN // 128        # 8 key tiles per batch
SCALE = HD ** -0.5


def build():
    nc = bacc.Bacc(None)
    xT_d = nc.declare_dram_parameter("xT", [D, T], BF16, isOutput=False)
    wqkv_d = nc.declare_dram_parameter("wqkv", [D, D3], BF16, isOutput=False)
    wproj_d = nc.declare_dram_parameter("wproj", [D, D], BF16, isOutput=False)
    bqk_d = nc.declare_dram_parameter("bqk", [128, 12], F32, isOutput=False)
    bv_d = nc.declare_dram_parameter("bv", [1, D], BF16, isOutput=False)
    bproj_d = nc.declare_dram_parameter("bproj", [1, D], BF16, isOutput=False)
    out_d = nc.declare_dram_parameter("out", [T, D], BF16, isOutput=True)

    with tile.TileContext(nc) as tc:
        with (
            nc.allow_low_precision(reason="bf16 attention pipeline"),
            tc.tile_pool(name="const", bufs=1) as cpool,
            tc.tile_pool(name="xu", bufs=2) as xupool,
            tc.tile_pool(name="qk", bufs=2) as qkpool,
            tc.tile_pool(name="vsb", bufs=2) as vpool,
            tc.tile_pool(name="esb", bufs=4) as epool,
            tc.tile_pool(name="stg", bufs=6) as spool,
            tc.tile_pool(name="gat", bufs=2) as gpool,
            tc.tile_pool(name="gsh", bufs=3) as gspool,
            tc.tile_pool(name="bsb", bufs=2) as bpool,
            tc.tile_pool(name="osb", bufs=3) as opool,
            tc.tile_pool(name="psS", bufs=2, space="PSUM") as psS,
            tc.tile_pool(name="psU", bufs=2, space="PSUM") as psU,
            tc.tile_pool(name="psQ", bufs=2, space="PSUM") as psQ,
        ):
            # ---- DMAs in priority order ----
            bqk = cpool.tile([128, 12], F32, tag="bqk")
            nc.sync.dma_start(bqk[:], bqk_d.ap())

            xTb = [None, None]
            xT_src = xT_d.ap().rearrange("(ko p) n -> p ko n", p=128)
            xTb[0] = xupool.tile([128, KT, TN], BF16, tag="x", name="xT0")
            for k in range(KT):
                nc.sync.dma_start(
                    xTb[0][:, k : k + 1, :], xT_src[:, k : k + 1, 0:TN]
                )

            # wqkv lands column-group-sliced so attention pair 0 (Q cols
            # 0:256 via m=0, K cols 768:1024 via m=6) and the first V tiles
            # (cols 1536:1920) are fed as early as possible.
            wqkv = cpool.tile([128, KT, D3], BF16, tag="wqkv")
            wqkv_src = wqkv_d.ap().rearrange("(ko p) n -> p ko n", p=128)

            def wqkv_cols(c0, c1):
                for k in range(KT):
                    nc.sync.dma_start(
                        wqkv[:, k : k + 1, c0:c1],
                        wqkv_src[:, k : k + 1, c0:c1],
                    )

            wqkv_cols(0, 256)        # Q pairs 0-1
            wqkv_cols(768, 1024)     # K pairs 0-1
            bv1 = cpool.tile([1, D], BF16, tag="bv1")
            nc.sync.dma_start(bv1[:], bv_d.ap())
            wqkv_cols(1536, 1920)    # V head-block nh=0
            wqkv_cols(256, 768)      # Q pairs 2-5
            wqkv_cols(1024, 1536)    # K pairs 2-5
            wqkv_cols(1920, 2304)    # V head-block nh=1
            bproj1 = cpool.tile([1, D], BF16, tag="bproj1")
            nc.sync.dma_start(bproj1[:], bproj_d.ap())
            wproj = cpool.tile([128, KT, D], BF16, tag="wproj")
            nc.sync.dma_start(
                wproj[:], wproj_d.ap().rearrange("(ko p) n -> p ko n", p=128)
            )
            xTb[1] = xupool.tile([128, KT, TN], BF16, tag="x", name="xT1")
            for k in range(KT):
                nc.sync.dma_start(
                    xTb[1][:, k : k + 1, :], xT_src[:, k : k + 1, TN : 2 * TN]
                )

            # ---- warmup: zeroed matmuls bridge the DMA wait so the PE
            # HAM clock-gate is warm when real work starts ----
            zt = cpool.tile([128, 640], BF16, tag="zt")
            nc.vector.memset(zt[:], 0.0)
            for _ in range(12):
                ps = psQ.tile([128, 512], F32, tag="ps")
                nc.tensor.matmul(
                    ps[:], zt[:, 0:128], zt[:, 128:640], start=True, stop=True
                )

            bvb = cpool.tile([128, D], BF16, tag="bvb")
            nc.gpsimd.partition_broadcast(bvb[:], bv1[:])
            bprojb = cpool.tile([128, D], BF16, tag="bprojb")
            nc.gpsimd.partition_broadcast(bprojb[:], bproj1[:])

            # ---- per-batch tiles (created lazily) ----
            qT = [None, None]
            kT = [None, None]
            vsb = [None, None]
            uT = [None, None]

            def make_batch_tiles(b):
                qT[b] = qkpool.tile([128, NPAIR, TN], BF16, tag="qT",
                                    name=f"qT{b}")
                kT[b] = qkpool.tile([128, NPAIR, TN], BF16, tag="kT",
                                    name=f"kT{b}")
                uT[b] = xupool.tile([128, KT, TN], BF16, tag="u",
                                    name=f"uT{b}")

            # ---- filler queue machinery ----
            # each item: (closure, tag_or_None); tag marks completion of a
            # unit of work other emission must wait for (emission order is
            # the only dependency mechanism — a read emitted before its
            # producer would silently read stale data).
            from collections import deque

            queue = deque()
            emitted = set()

            def pop(n=1):
                for _ in range(n):
                    if not queue:
                        return
                    fn, tag = queue.popleft()
                    fn()
                    if tag is not None:
                        emitted.add(tag)

            def require(tag):
                while tag not in emitted and queue:
                    pop(1)
                assert tag in emitted, f"filler ordering bug: {tag}"

            def qk_items(b, hp):
                """24 matmul closures computing Q^T,K^T feature tiles of
                head-pair hp; DVE bias-add evac attached to each 6th."""
                items = []
                for m in (hp, hp + 6):
                    for ih in range(2):
                        st = {}

                        def mk(k, m=m, ih=ih, st=st):
                            def f():
                                if k == 0:
                                    st["ps"] = psQ.tile([128, 512], F32,
                                                        tag="ps", name="ps")
                                dst = qT[b] if m < 6 else kT[b]
                                nc.tensor.matmul(
                                    st["ps"][:],
                                    wqkv[:, k, m * 128 : (m + 1) * 128],
                                    xTb[b][:, k, ih * 512 : (ih + 1) * 512],
                                    start=(k == 0),
                                    stop=(k == KT - 1),
                                )
                                if k == KT - 1:
                                    nc.vector.tensor_scalar_add(
                                        dst[:, hp, ih * 512 : (ih + 1) * 512],
                                        st["ps"][:],
                                        bqk[:, m : m + 1],
                                    )
                            return f

                        for k in range(KT):
                            items.append((mk(k), None))
                items[-1] = (items[-1][0], ("qk", b, hp))
                return items

            def v_items(b, nh, ts):
                """V tiles (token-major, + b_v) for key tiles ts; tag per
                completed (b, nh, t) tile."""
                items = []
                for t in ts:
                    st = {}

                    def mk(k, t=t, st=st):
                        def f():
                            if k == 0:
                                st["ps"] = psQ.tile([128, 384], F32,
                                                    tag="ps", name="ps")
                            nc.tensor.matmul(
                                st["ps"][:],
                                xTb[b][:, k, t * 128 : (t + 1) * 128],
                                wqkv[
                                    :, k,
                                    2 * D + nh * 384 : 2 * D + (nh + 1) * 384,
                                ],
                                start=(k == 0),
                                stop=(k == KT - 1),
                            )
                            if k == KT - 1:
                                nc.vector.tensor_add(
                                    vsb[b][:, t, nh * 6 : (nh + 1) * 6, 0:HD],
                                    st["ps"][:],
                                    bvb[:, nh * 384 : (nh + 1) * 384],
                                )
                        return f

                    for k in range(KT):
                        items.append(
                            (mk(k), ("v", b, nh, t) if k == KT - 1 else None)
                        )
                return items

            def vsb_init_item(b):
                def f():
                    vsb[b] = vpool.tile([128, NJT, H, HD + 1], BF16,
                                        tag="v", name=f"v{b}")
                    nc.vector.memset(vsb[b][:, :, :, HD : HD + 1], 1.0)
                return (f, ("vsb", b))

            def proj_items(b, ts):
                """Projection tiles for batch b tokens ts (needs uT[b]
                fully normalized); bf16 out + DMA."""
                tok0 = b * TN
                items = []
                for t in ts:
                    for nh in range(2):
                        st = {}

                        def mk(k, t=t, nh=nh, st=st):
                            def f():
                                if k == 0:
                                    st["ps"] = psQ.tile([128, 384], F32,
                                                        tag="ps", name="ps")
                                nc.tensor.matmul(
                                    st["ps"][:],
                                    uT[b][:, k, t * 128 : (t + 1) * 128],
                                    wproj[:, k, nh * 384 : (nh + 1) * 384],
                                    start=(k == 0),
                                    stop=(k == KT - 1),
                                )
                                if k == KT - 1:
                                    ot = opool.tile([128, 384], BF16,
                                                    tag="o", name="ot")
                                    nc.vector.tensor_add(
                                        ot[:], st["ps"][:],
                                        bprojb[:, nh * 384 : (nh + 1) * 384],
                                    )
                                    nc.sync.dma_start(
                                        out_d.ap()[
                                            tok0 + t * 128 : tok0 + (t + 1) * 128,
                                            nh * 384 : (nh + 1) * 384,
                                        ],
                                        ot[:],
                                    )
                            return f

                        for k in range(KT):
                            items.append((mk(k), None))
                items[-1] = (items[-1][0], ("proj", b, ts[-1]))
                return items

            # ---- normalization ----
            def emit_norm(b, hp, stages):
                """Batched normalization of one head-pair (4 (ih,h) stages):
                one reciprocal covers all four denominator rows (at
                partition bases 0/32/64/96), then GpSimd broadcast + DVE
                multiply. Runs off the critical path."""
                g = gpool.tile([97, 512], F32, tag="g")
                nc.vector.memset(g[:], 1.0)
                for (ih, h, ust) in stages:
                    nc.vector.tensor_copy(
                        g[32 * (2 * ih + h) : 32 * (2 * ih + h) + 1, :],
                        ust[HD : HD + 1, :],
                    )
                rc = gpool.tile([97, 512], F32, tag="g")
                # denominators are sums of exps (positive, ~e^±8): safe for
                # the fast approx (~18 correct bits, 5x cheaper), and the
                # result feeds bf16 anyway.
                nc.vector.reciprocal_approx_fast(rc[:], g[:])
                shifted = {0: rc}
                for idx in (1, 2, 3):
                    tsh = gspool.tile([1, 512], F32, tag="gs",
                                      name=f"gs{idx}")
                    nc.vector.tensor_copy(
                        tsh[0:1, :], rc[32 * idx : 32 * idx + 1, :]
                    )
                    shifted[idx] = tsh
                for (ih, h, ust) in stages:
                    idx = 2 * ih + h
                    rb = bpool.tile([128, 512], F32, tag="rb")
                    nc.gpsimd.partition_broadcast(rb[:], shifted[idx][0:1, :])
                    usl = uT[b][
                        h * 64 : (h + 1) * 64, hp, ih * 512 : ih * 512 + 512
                    ]
                    nc.vector.tensor_mul(usl, ust[0:HD, :], rb[0:HD, :])

            def emit_norm_fast(b, hp, stages):
                """Endgame normalization of a 2-stage (single ih) subset —
                same structure as emit_norm; latency is short because only
                two denominator rows ride the reciprocal."""
                emit_norm(b, hp, stages)

            # ---- paced attention pair ----
            pending = [None]  # (b, hp, stages) awaiting normalization

            def emit_attn(b, hp, endgame=False):
                if pending[0] is not None:
                    emit_norm(*pending[0])
                    pending[0] = None
                nh = 0 if hp < 3 else 1
                stages = []
                for ih in range(2):
                    i0 = ih * 512
                    pu = [
                        psU.tile([HD + 1, 512], F32, tag="pu", name=f"pu{h}")
                        for h in range(2)
                    ]
                    e_tiles = []
                    for jt in range(NJT):
                        ps = psS.tile([128, 1024], F32, tag="s")
                        for h in range(2):
                            nc.tensor.matmul(
                                ps[:, h * 512 : (h + 1) * 512],
                                kT[b][h * 64 : (h + 1) * 64, hp,
                                      jt * 128 : (jt + 1) * 128],
                                qT[b][h * 64 : (h + 1) * 64, hp, i0 : i0 + 512],
                            )
                        e = epool.tile([128, 1024], BF16, tag="e")
                        nc.scalar.activation(e[:], ps[:], AF.Exp, scale=SCALE)
                        e_tiles.append(e)
                        if jt == 0:
                            pop(4)
                        else:
                            require(("v", b, nh, jt - 1))
                            for h in range(2):
                                nc.tensor.matmul(
                                    pu[h][:],
                                    vsb[b][:, jt - 1, 2 * hp + h, :],
                                    e_tiles[jt - 1][:, h * 512 : (h + 1) * 512],
                                    start=(jt == 1),
                                    stop=False,
                                )
                            pop(2)
                    require(("v", b, nh, NJT - 1))
                    for h in range(2):
                        nc.tensor.matmul(
                            pu[h][:],
                            vsb[b][:, NJT - 1, 2 * hp + h, :],
                            e_tiles[NJT - 1][:, h * 512 : (h + 1) * 512],
                            start=False,
                            stop=True,
                        )
                    ih_stages = []
                    for h in range(2):
                        ust = spool.tile([HD + 1, 512], F32, tag="ust")
                        nc.vector.tensor_copy(ust[:], pu[h][:])
                        ih_stages.append((ih, h, ust))
                    stages += ih_stages
                    if endgame and ih == 0:
                        # normalize ih0 of the final pair right away (fast
                        # path) and queue the first half of the final
                        # projection as fillers for ih1.
                        emit_norm_fast(b, hp, ih_stages)
                        queue.extend(proj_items(b, [0, 1, 2, 3]))
                if endgame:
                    emit_norm_fast(b, hp, stages[2:])
                else:
                    pending[0] = (b, hp, stages)

            # =================================================================
            # batch 0
            # =================================================================
            make_batch_tiles(0)
            f, tag = vsb_init_item(0)
            f()
            emitted.add(tag)
            for fn, tag in qk_items(0, 0):
                fn()
                if tag:
                    emitted.add(tag)
            for fn, tag in v_items(0, 0, [0, 1, 2, 3]):
                fn()
                if tag:
                    emitted.add(tag)

            queue.extend(v_items(0, 0, [4, 5, 6, 7]))
            queue.extend(qk_items(0, 1))
            queue.extend(qk_items(0, 2))
            queue.extend(v_items(0, 1, list(range(NJT))))
            queue.extend(qk_items(0, 3))
            queue.extend(qk_items(0, 4))
            queue.extend(qk_items(0, 5))
            # cross-batch lead-in: batch 1's first QK pair + early V tiles
            # fill batch 0's last attention pairs.
            def b1_tiles_item():
                def f():
                    make_batch_tiles(1)
                return (f, ("tiles", 1))

            queue.append(b1_tiles_item())
            queue.append(vsb_init_item(1))
            queue.extend(qk_items(1, 0))
            queue.extend(v_items(1, 0, [0, 1, 2, 3]))
            queue.extend(qk_items(1, 1))
            queue.extend(v_items(1, 0, [4, 5, 6, 7]))
            queue.extend(qk_items(1, 2))

            for hp in range(NPAIR):
                require(("qk", 0, hp))
                emit_attn(0, hp)

            # =================================================================
            # batch 1
            # =================================================================
            require(("tiles", 1))
            require(("vsb", 1))
            if pending[0] is not None:
                emit_norm(*pending[0])
                pending[0] = None

            queue.extend(v_items(1, 1, list(range(NJT))))
            queue.extend(qk_items(1, 3))
            queue.extend(proj_items(0, [0, 1, 2, 3]))
            queue.extend(qk_items(1, 4))
            queue.extend(proj_items(0, [4, 5, 6, 7]))
            queue.extend(qk_items(1, 5))

            for hp in range(NPAIR):
                require(("qk", 1, hp))
                if hp == NPAIR - 1:
                    # the tail must be only the final projection: drain
                    # everything else (incl. batch 0's proj) first.
                    require(("proj", 0, 7))
                emit_attn(1, hp, endgame=(hp == NPAIR - 1))

            # final projection, second token half (first half was queued
            # during the endgame pair's ih=1)
            queue.extend(proj_items(1, [4, 5, 6, 7]))
            while queue:
                pop(1)

    nc.compile()
    return nc


_NC_CACHE = None


def _get_nc():
    global _NC_CACHE
    if _NC_CACHE is None:
        _NC_CACHE = build()
    return _NC_CACHE


def _prep_core_inputs(x_c, w_qkv, b_qkv, w_proj, b_proj):
    """Host-side layout prep for one core. x_c: [2, 1024, 768]."""
    xT = np.ascontiguousarray(x_c.reshape(T, D).T).astype(np.float32)
    bqk = np.ascontiguousarray(b_qkv[: 12 * 128].reshape(12, 128).T)
    import ml_dtypes

    bf = ml_dtypes.bfloat16
    return {
        "xT": np.ascontiguousarray(xT.astype(bf)),
        "wqkv": np.ascontiguousarray(w_qkv.astype(bf)),
        "wproj": np.ascontiguousarray(w_proj.astype(bf)),
        "bqk": bqk.astype(np.float32),
        "bv": np.ascontiguousarray(b_qkv[2 * D :].reshape(1, D).astype(bf)),
        "bproj": np.ascontiguousarray(b_proj.reshape(1, D).astype(bf)),
    }


def kernel(x, w_qkv, b_qkv, w_proj, b_proj):
    x = np.asarray(x, dtype=np.float32)
    w_qkv = np.asarray(w_qkv, dtype=np.float32)
    b_qkv = np.asarray(b_qkv, dtype=np.float32)
    w_proj = np.asarray(w_proj, dtype=np.float32)
    b_proj = np.asarray(b_proj, dtype=np.float32)
    B, N, Dd = x.shape
    assert (B, N, Dd) == (16, 1024, 768)

    nc = _get_nc()
    in_maps = [
        _prep_core_inputs(x[2 * c : 2 * c + 2], w_qkv, b_qkv, w_proj, b_proj)
        for c in range(8)
    ]
    res = run_bass_kernel_spmd(nc, in_maps, core_ids=list(range(8)))
    out = np.empty((B, N, Dd), dtype=np.float32)
    for c in range(8):
        out[2 * c : 2 * c + 2] = (
            res.results[c]["out"].astype(np.float32).reshape(2, N, Dd)
        )
    kernel.last_results = res
    return out


# revision 22
# speedup vs baseline: 1.2307x; 1.0454x over previous
"""Multi-head attention kernel for Trainium2, data-parallel over 8 NeuronCores.

Problem: B=16, N=1024, D=768, H=12 heads (hd=64), fp32 I/O.
  qkv = x @ w_qkv + b_qkv ; attention ; out = attn_out @ w_proj + b_proj

Sharding: batch data-parallel — core c handles batches [2c, 2c+2); weights
replicated. Inside each core, the two batches are processed sequentially.

Layout strategy (matmuls in bf16, ~7e-3 rel overall):
  - host pre-transposes x to xT [768, T] so the in-feature contraction has
    features on partitions for both operands.
  - Q^T, K^T computed feature-major [768, N]: lhsT = w_qkv cols, rhs = xT.
    A 128-row feature tile holds a PAIR of heads (2x64) -> scores matmuls
    for the two heads share one PSUM tile via row packing (K=64).
  - V computed token-major [N, 768], stored bf16 with a ones column
    appended per head (v_ext [128, 65]).
  - scores^T tiles [128 j, 512 q] per head -> one ACT exp op [128, 1024]
    covers both heads of a pair (softmax scale folded into exp's scale).
  - U^T = sum_j exp * v_ext accumulates in PSUM [65, 512]; row 64 is the
    softmax denominator (ones column) — no separate reduction needed.
  - normalize: reciprocal (DVE) -> partition_broadcast (GpSimd) -> multiply
    (DVE), + b_v per-partition. b_q/b_k added at Q^T/K^T evacuation.
  - proj: lhsT = attn^T tile, rhs = w_proj; + b_proj via broadcast add.
    Output lands token-major [T, 768] in bf16 (upcast on host).

Schedule (the point of this revision):
  - ACT exp (1063ns per [128,1024]) paces each attention step; the two
    scores + two attnV matmuls only cover 852ns. A queue of single-matmul
    "filler" closures (QK / V / proj tiles) is popped between attention
    steps so TensorE never idles waiting for exp.
  - DMA priority order: bqk, xT(b0), wqkv k-slices, bv, bproj, wproj,
    xT(b1) — the first QK matmul can start as soon as wqkv slice k=0
    lands (~7us) instead of after all inputs (~32us).
  - Dummy matmuls on a zeroed tile bridge the initial DMA wait so the PE
    HAM clock-gate warms up (and stays warm — no 1.2GHz cold periods).
  - ones column of v_ext via on-chip memset (the DMA version generated
    24576 2-byte packets that polluted every DMA queue).
  - Endgame: the last pair's normalization runs via ACT reciprocal +
    TensorE ones-broadcast (short latency, engines that are idle then),
    and the final batch's projection is split into two token halves so
    the first half overlaps the last pair's ih=1 attention.
"""

import contextlib
import ctypes
import os
import sys
import types

import numpy as np

# ---------------------------------------------------------------------------
# NTFF profiling shim: bass_utils's trace path imports
# antenv.axon_hooks.get_axon_ntff_profile_hook, which this container's antenv
# lacks. Register a ctypes-based equivalent so BASS_TRACE=1 works. Harmless
# if tracing is never requested.
# ---------------------------------------------------------------------------


def _install_ntff_shim():
    if "antenv.axon_hooks" in sys.modules:
        return
    so_path = "/opt/axon/libaxon_pjrt.so"
    hook = None
    try:
        lib = ctypes.CDLL(so_path)
        if hasattr(lib, "axon_start_nrt_profile"):
            lib.axon_start_nrt_profile.argtypes = [
                ctypes.POINTER(ctypes.c_int64),
                ctypes.c_size_t,
            ]
            lib.axon_start_nrt_profile.restype = ctypes.c_int64
            lib.axon_stop_nrt_profile.argtypes = [ctypes.c_char_p]
            lib.axon_stop_nrt_profile.restype = ctypes.c_int64

            @contextlib.contextmanager
            def _hook(output_dir, device_ids):
                import jax

                jax.devices()
                if device_ids:
                    ids = (ctypes.c_int64 * len(device_ids))(*device_ids)
                    rc = lib.axon_start_nrt_profile(ids, len(device_ids))
                else:
                    rc = lib.axon_start_nrt_profile(None, 0)
                if rc != 0:
                    raise RuntimeError(f"axon_start_nrt_profile rc={rc}")
                try:
                    yield
                finally:
                    n = lib.axon_stop_nrt_profile(str(output_dir).encode())
                    print(f"ntff profile: {n} file(s) in {output_dir}",
                          file=sys.stderr)

            hook = _hook
    except OSError:
        pass
    mod = types.ModuleType("antenv.axon_hooks")
    mod.get_axon_ntff_profile_hook = lambda: hook
    mod.set_axon_ntff_profile_hook = lambda h: None
    sys.modules["antenv.axon_hooks"] = mod


_install_ntff_shim()

import concourse.bacc as bacc  # noqa: E402
import concourse.mybir as mybir  # noqa: E402
import concourse.tile as tile  # noqa: E402
from concourse.bass_utils import run_bass_kernel_spmd  # noqa: E402

F32 = mybir.dt.float32
F32R = mybir.dt.float32r
BF16 = mybir.dt.bfloat16
AF = mybir.ActivationFunctionType

# Problem constants (per core)
NB = 2        # batches per core
TN = 1024     # tokens per batch
T = NB * TN   # tokens per core
D = 768
H = 12
HD = 64
D3 = 3 * D
KT = D // 128          # 6 contraction tiles
NPAIR = H // 2         # 6 head pairs
NJT = TN // 128        # 8 key tiles per batch
SCALE = HD ** -0.5


def build():
    nc = bacc.Bacc(None)
    xT_d = nc.declare_dram_parameter("xT", [D, T], BF16, isOutput=False)
    wqkv_d = nc.declare_dram_parameter("wqkv", [D, D3], BF16, isOutput=False)
    wproj_d = nc.declare_dram_parameter("wproj", [D, D], BF16, isOutput=False)
    bqk_d = nc.declare_dram_parameter("bqk", [128, 12], F32, isOutput=False)
    bv_d = nc.declare_dram_parameter("bv", [1, D], BF16, isOutput=False)
    bproj_d = nc.declare_dram_parameter("bproj", [1, D], BF16, isOutput=False)
    out_d = nc.declare_dram_parameter("out", [T, D], BF16, isOutput=True)

    with tile.TileContext(nc) as tc:
        with (
            nc.allow_low_precision(reason="bf16 attention pipeline"),
            tc.tile_pool(name="const", bufs=1) as cpool,
            tc.tile_pool(name="xu", bufs=2) as xupool,
            tc.tile_pool(name="qk", bufs=2) as qkpool,
            tc.tile_pool(name="vsb", bufs=2) as vpool,
            tc.tile_pool(name="esb", bufs=4) as epool,
            tc.tile_pool(name="stg", bufs=6) as spool,
            tc.tile_pool(name="gat", bufs=2) as gpool,
            tc.tile_pool(name="gsh", bufs=3) as gspool,
            tc.tile_pool(name="bsb", bufs=2) as bpool,
            tc.tile_pool(name="osb", bufs=3) as opool,
            tc.tile_pool(name="psS", bufs=2, space="PSUM") as psS,
            tc.tile_pool(name="psU", bufs=2, space="PSUM") as psU,
            tc.tile_pool(name="psQ", bufs=2, space="PSUM") as psQ,
        ):
            # ---- DMAs in priority order ----
            bqk = cpool.tile([128, 12], F32, tag="bqk")
            nc.sync.dma_start(bqk[:], bqk_d.ap())

            xTb = [None, None]
            xT_src = xT_d.ap().rearrange("(ko p) n -> p ko n", p=128)
            xTb[0] = xupool.tile([128, KT, TN], BF16, tag="x", name="xT0")
            for k in range(KT):
                nc.sync.dma_start(
                    xTb[0][:, k : k + 1, :], xT_src[:, k : k + 1, 0:TN]
                )

            # wqkv lands column-group-sliced so attention pair 0 (Q cols
            # 0:256 via m=0, K cols 768:1024 via m=6) and the first V tiles
            # (cols 1536:1920) are fed as early as possible.
            wqkv = cpool.tile([128, KT, D3], BF16, tag="wqkv")
            wqkv_src = wqkv_d.ap().rearrange("(ko p) n -> p ko n", p=128)

            def wqkv_cols(c0, c1):
                for k in range(KT):
                    nc.sync.dma_start(
                        wqkv[:, k : k + 1, c0:c1],
                        wqkv_src[:, k : k + 1, c0:c1],
                    )

            wqkv_cols(0, 256)        # Q pairs 0-1
            wqkv_cols(768, 1024)     # K pairs 0-1
            bv1 = cpool.tile([1, D], BF16, tag="bv1")
            nc.sync.dma_start(bv1[:], bv_d.ap())
            wqkv_cols(1536, 1920)    # V head-block nh=0
            wqkv_cols(256, 768)      # Q pairs 2-5
            wqkv_cols(1024, 1536)    # K pairs 2-5
            wqkv_cols(1920, 2304)    # V head-block nh=1
            bproj1 = cpool.tile([1, D], BF16, tag="bproj1")
            nc.sync.dma_start(bproj1[:], bproj_d.ap())
            wproj = cpool.tile([128, KT, D], BF16, tag="wproj")
            nc.sync.dma_start(
                wproj[:], wproj_d.ap().rearrange("(ko p) n -> p ko n", p=128)
            )
            xTb[1] = xupool.tile([128, KT, TN], BF16, tag="x", name="xT1")
            for k in range(KT):
                nc.sync.dma_start(
                    xTb[1][:, k : k + 1, :], xT_src[:, k : k + 1, TN : 2 * TN]
                )

            # ---- warmup: zeroed matmuls bridge the DMA wait so the PE
            # HAM clock-gate is warm when real work starts ----
            zt = cpool.tile([128, 640], BF16, tag="zt")
            nc.vector.memset(zt[:], 0.0)
            for _ in range(12):
                ps = psQ.tile([128, 512], F32, tag="ps")
                nc.tensor.matmul(
                    ps[:], zt[:, 0:128], zt[:, 128:640], start=True, stop=True
                )

            bvb = cpool.tile([128, D], BF16, tag="bvb")
            nc.gpsimd.partition_broadcast(bvb[:], bv1[:])
            bprojb = cpool.tile([128, D], BF16, tag="bprojb")
            nc.gpsimd.partition_broadcast(bprojb[:], bproj1[:])

            # ---- per-batch tiles (created lazily) ----
            qT = [None, None]
            kT = [None, None]
            vsb = [None, None]
            uT = [None, None]

            def make_batch_tiles(b):
                qT[b] = qkpool.tile([128, NPAIR, TN], BF16, tag="qT",
                                    name=f"qT{b}")
                kT[b] = qkpool.tile([128, NPAIR, TN], BF16, tag="kT",
                                    name=f"kT{b}")
                uT[b] = xupool.tile([128, KT, TN], BF16, tag="u",
                                    name=f"uT{b}")

            # ---- filler queue machinery ----
            # each item: (closure, tag_or_None); tag marks completion of a
            # unit of work other emission must wait for (emission order is
            # the only dependency mechanism — a read emitted before its
            # producer would silently read stale data).
            from collections import deque

            queue = deque()
            emitted = set()

            def pop(n=1):
                for _ in range(n):
                    if not queue:
                        return
                    fn, tag = queue.popleft()
                    fn()
                    if tag is not None:
                        emitted.add(tag)

            def require(tag):
                while tag not in emitted and queue:
                    pop(1)
                assert tag in emitted, f"filler ordering bug: {tag}"

            def qk_items(b, hp):
                """24 matmul closures computing Q^T,K^T feature tiles of
                head-pair hp; DVE bias-add evac attached to each 6th."""
                items = []
                for m in (hp, hp + 6):
                    for ih in range(2):
                        st = {}

                        def mk(k, m=m, ih=ih, st=st):
                            def f():
                                if k == 0:
                                    st["ps"] = psQ.tile([128, 512], F32,
                                                        tag="ps", name="ps")
                                dst = qT[b] if m < 6 else kT[b]
                                nc.tensor.matmul(
                                    st["ps"][:],
                                    wqkv[:, k, m * 128 : (m + 1) * 128],
                                    xTb[b][:, k, ih * 512 : (ih + 1) * 512],
                                    start=(k == 0),
                                    stop=(k == KT - 1),
                                )
                                if k == KT - 1:
                                    nc.vector.tensor_scalar_add(
                                        dst[:, hp, ih * 512 : (ih + 1) * 512],
                                        st["ps"][:],
                                        bqk[:, m : m + 1],
                                    )
                            return f

                        for k in range(KT):
                            items.append((mk(k), None))
                items[-1] = (items[-1][0], ("qk", b, hp))
                return items

            def v_items(b, nh, ts):
                """V tiles (token-major, + b_v) for key tiles ts; tag per
                completed (b, nh, t) tile."""
                items = []
                for t in ts:
                    st = {}

                    def mk(k, t=t, st=st):
                        def f():
                            if k == 0:
                                st["ps"] = psQ.tile([128, 384], F32,
                                                    tag="ps", name="ps")
                            nc.tensor.matmul(
                                st["ps"][:],
                                xTb[b][:, k, t * 128 : (t + 1) * 128],
                                wqkv[
                                    :, k,
                                    2 * D + nh * 384 : 2 * D + (nh + 1) * 384,
                                ],
                                start=(k == 0),
                                stop=(k == KT - 1),
                            )
                            if k == KT - 1:
                                nc.vector.tensor_add(
                                    vsb[b][:, t, nh * 6 : (nh + 1) * 6, 0:HD],
                                    st["ps"][:],
                                    bvb[:, nh * 384 : (nh + 1) * 384],
                                )
                        return f

                    for k in range(KT):
                        items.append(
                            (mk(k), ("v", b, nh, t) if k == KT - 1 else None)
                        )
                return items

            def vsb_init_item(b):
                def f():
                    vsb[b] = vpool.tile([128, NJT, H, HD + 1], BF16,
                                        tag="v", name=f"v{b}")
                    nc.vector.memset(vsb[b][:, :, :, HD : HD + 1], 1.0)
                return (f, ("vsb", b))

            def proj_items(b, ts):
                """Projection tiles for batch b tokens ts (needs uT[b]
                fully normalized); bf16 out + DMA."""
                tok0 = b * TN
                items = []
                for t in ts:
                    for nh in range(2):
                        st = {}

                        def mk(k, t=t, nh=nh, st=st):
                            def f():
                                if k == 0:
                                    st["ps"] = psQ.tile([128, 384], F32,
                                                        tag="ps", name="ps")
                                nc.tensor.matmul(
                                    st["ps"][:],
                                    uT[b][:, k, t * 128 : (t + 1) * 128],
                                    wproj[:, k, nh * 384 : (nh + 1) * 384],
                                    start=(k == 0),
                                    stop=(k == KT - 1),
                                )
                                if k == KT - 1:
                                    ot = opool.tile([128, 384], BF16,
                                                    tag="o", name="ot")
                                    nc.vector.tensor_add(
                                        ot[:], st["ps"][:],
                                        bprojb[:, nh * 384 : (nh + 1) * 384],
                                    )
                                    nc.sync.dma_start(
                                        out_d.ap()[
                                            tok0 + t * 128 : tok0 + (t + 1) * 128,
                                            nh * 384 : (nh + 1) * 384,
                                        ],
                                        ot[:],
                                    )
                            return f

                        for k in range(KT):
                            items.append((mk(k), None))
                items[-1] = (items[-1][0], ("proj", b, ts[-1]))
                return items

            # ---- normalization ----
            def emit_norm(b, hp, stages):
                """Batched normalization of one head-pair (4 (ih,h) stages):
                one reciprocal covers all four denominator rows (at
                partition bases 0/32/64/96), then GpSimd broadcast + DVE
                multiply. Runs off the critical path."""
                g = gpool.tile([97, 512], F32, tag="g")
                nc.vector.memset(g[:], 1.0)
                for (ih, h, ust) in stages:
                    nc.vector.tensor_copy(
                        g[32 * (2 * ih + h) : 32 * (2 * ih + h) + 1, :],
                        ust[HD : HD + 1, :],
                    )
                rc = gpool.tile([97, 512], F32, tag="g")
                # denominators are sums of exps (positive, ~e^±8): safe for
                # the fast approx (~18 correct bits, 5x cheaper), and the
                # result feeds bf16 anyway.
                nc.vector.reciprocal_approx_fast(rc[:], g[:])
                shifted = {0: rc}
                for idx in (1, 2, 3):
                    tsh = gspool.tile([1, 512], F32, tag="gs",
                                      name=f"gs{idx}")
                    nc.vector.tensor_copy(
                        tsh[0:1, :], rc[32 * idx : 32 * idx + 1, :]
                    )
                    shifted[idx] = tsh
                for (ih, h, ust) in stages:
                    idx = 2 * ih + h
                    rb = bpool.tile([128, 512], F32, tag="rb")
                    nc.gpsimd.partition_broadcast(rb[:], shifted[idx][0:1, :])
                    usl = uT[b][
                        h * 64 : (h + 1) * 64, hp, ih * 512 : ih * 512 + 512
                    ]
                    nc.vector.tensor_mul(usl, ust[0:HD, :], rb[0:HD, :])

            def emit_norm_fast(b, hp, stages):
                """Endgame normalization of a 2-stage (single ih) subset —
                same structure as emit_norm; latency is short because only
                two denominator rows ride the reciprocal."""
                emit_norm(b, hp, stages)

            # ---- paced attention pair ----
            pending = [None]  # (b, hp, stages) awaiting normalization

            def emit_attn(b, hp, endgame=False):
                nh = 0 if hp < 3 else 1
                stages = []
                for ih in range(2):
                    i0 = ih * 512
                    pu = [
                        psU.tile([HD + 1, 512], F32, tag="pu", name=f"pu{h}")
                        for h in range(2)
                    ]
                    e_tiles = []
                    for jt in range(NJT):
                        ps = psS.tile([128, 1024], F32, tag="s")
                        for h in range(2):
                            nc.tensor.matmul(
                                ps[:, h * 512 : (h + 1) * 512],
                                kT[b][h * 64 : (h + 1) * 64, hp,
                                      jt * 128 : (jt + 1) * 128],
                                qT[b][h * 64 : (h + 1) * 64, hp, i0 : i0 + 512],
                            )
                        e = epool.tile([128, 1024], BF16, tag="e")
                        nc.scalar.activation(e[:], ps[:], AF.Exp, scale=SCALE)
                        e_tiles.append(e)
                        if jt == 0:
                            pop(4)
                        else:
                            require(("v", b, nh, jt - 1))
                            for h in range(2):
                                nc.tensor.matmul(
                                    pu[h][:],
                                    vsb[b][:, jt - 1, 2 * hp + h, :],
                                    e_tiles[jt - 1][:, h * 512 : (h + 1) * 512],
                                    start=(jt == 1),
                                    stop=False,
                                )
                            pop(2)
                        if jt == 2 and pending[0] is not None:
                            # previous pair's normalization, one ih half at a
                            # time, mid-pair — keeps the DVE burst away from
                            # the boundary drains that gate psQ reuse.
                            pb, php, pstages = pending[0]
                            emit_norm(pb, php, pstages[2 * ih : 2 * ih + 2])
                            if ih == 1:
                                pending[0] = None
                    require(("v", b, nh, NJT - 1))
                    for h in range(2):
                        nc.tensor.matmul(
                            pu[h][:],
                            vsb[b][:, NJT - 1, 2 * hp + h, :],
                            e_tiles[NJT - 1][:, h * 512 : (h + 1) * 512],
                            start=False,
                            stop=True,
                        )
                    ih_stages = []
                    for h in range(2):
                        ust = spool.tile([HD + 1, 512], F32, tag="ust")
                        nc.vector.tensor_copy(ust[:], pu[h][:])
                        ih_stages.append((ih, h, ust))
                    stages += ih_stages
                    pop(4)
                    if endgame and ih == 0:
                        # normalize ih0 of the final pair right away (fast
                        # path) and queue the first half of the final
                        # projection as fillers for ih1.
                        emit_norm_fast(b, hp, ih_stages)
                        queue.extend(proj_items(b, [0, 1, 2, 3]))
                if endgame:
                    emit_norm_fast(b, hp, stages[2:])
                else:
                    pending[0] = (b, hp, stages)

            # =================================================================
            # batch 0
            # =================================================================
            make_batch_tiles(0)
            f, tag = vsb_init_item(0)
            f()
            emitted.add(tag)
            for fn, tag in qk_items(0, 0):
                fn()
                if tag:
                    emitted.add(tag)
            for fn, tag in v_items(0, 0, [0, 1, 2, 3]):
                fn()
                if tag:
                    emitted.add(tag)

            queue.extend(v_items(0, 0, [4, 5, 6, 7]))
            queue.extend(qk_items(0, 1))
            queue.extend(qk_items(0, 2))
            queue.extend(v_items(0, 1, list(range(NJT))))
            queue.extend(qk_items(0, 3))
            queue.extend(qk_items(0, 4))
            queue.extend(qk_items(0, 5))
            # cross-batch lead-in: batch 1's first QK pair + early V tiles
            # fill batch 0's last attention pairs.
            def b1_tiles_item():
                def f():
                    make_batch_tiles(1)
                return (f, ("tiles", 1))

            queue.append(b1_tiles_item())
            queue.append(vsb_init_item(1))
            queue.extend(qk_items(1, 0))
            queue.extend(v_items(1, 0, [0, 1, 2, 3]))
            queue.extend(qk_items(1, 1))
            queue.extend(v_items(1, 0, [4, 5, 6, 7]))
            queue.extend(qk_items(1, 2))

            for hp in range(NPAIR):
                require(("qk", 0, hp))
                emit_attn(0, hp)

            # =================================================================
            # batch 1
            # =================================================================
            require(("tiles", 1))
            require(("vsb", 1))

            queue.extend(v_items(1, 1, list(range(NJT))))
            queue.extend(qk_items(1, 3))
            queue.extend(proj_items(0, [0, 1, 2, 3]))
            queue.extend(qk_items(1, 4))
            queue.extend(proj_items(0, [4, 5, 6, 7]))
            queue.extend(qk_items(1, 5))

            for hp in range(NPAIR):
                require(("qk", 1, hp))
                if hp == NPAIR - 1:
                    # the tail must be only the final projection: drain
                    # everything else (incl. batch 0's proj) first.
                    require(("proj", 0, 7))
                emit_attn(1, hp, endgame=(hp == NPAIR - 1))

            # final projection, second token half (first half was queued
            # during the endgame pair's ih=1)
            queue.extend(proj_items(1, [4, 5, 6, 7]))
            while queue:
                pop(1)

    nc.compile()
    return nc


_NC_CACHE = None


def _get_nc():
    global _NC_CACHE
    if _NC_CACHE is None:
        _NC_CACHE = build()
    return _NC_CACHE


def _prep_core_inputs(x_c, w_qkv, b_qkv, w_proj, b_proj):
    """Host-side layout prep for one core. x_c: [2, 1024, 768]."""
    xT = np.ascontiguousarray(x_c.reshape(T, D).T).astype(np.float32)
    bqk = np.ascontiguousarray(b_qkv[: 12 * 128].reshape(12, 128).T)
    import ml_dtypes

    bf = ml_dtypes.bfloat16
    return {
        "xT": np.ascontiguousarray(xT.astype(bf)),
        "wqkv": np.ascontiguousarray(w_qkv.astype(bf)),
        "wproj": np.ascontiguousarray(w_proj.astype(bf)),
        "bqk": bqk.astype(np.float32),
        "bv": np.ascontiguousarray(b_qkv[2 * D :].reshape(1, D).astype(bf)),
        "bproj": np.ascontiguousarray(b_proj.reshape(1, D).astype(bf)),
    }


def kernel(x, w_qkv, b_qkv, w_proj, b_proj):
    x = np.asarray(x, dtype=np.float32)
    w_qkv = np.asarray(w_qkv, dtype=np.float32)
    b_qkv = np.asarray(b_qkv, dtype=np.float32)
    w_proj = np.asarray(w_proj, dtype=np.float32)
    b_proj = np.asarray(b_proj, dtype=np.float32)
    B, N, Dd = x.shape
    assert (B, N, Dd) == (16, 1024, 768)

    nc = _get_nc()
    in_maps = [
        _prep_core_inputs(x[2 * c : 2 * c + 2], w_qkv, b_qkv, w_proj, b_proj)
        for c in range(8)
    ]
    res = run_bass_kernel_spmd(nc, in_maps, core_ids=list(range(8)))
    out = np.empty((B, N, Dd), dtype=np.float32)
    for c in range(8):
        out[2 * c : 2 * c + 2] = (
            res.results[c]["out"].astype(np.float32).reshape(2, N, Dd)
        )
    kernel.last_results = res
    return out
